# revision 43
# baseline (speedup 1.0000x reference)
"""Trainium2 Bass kernel for sparse causal attention (nn_CausalAttentionKV).

Reference computation (fp32, single device):
    q_all = x @ Wq + bq ; k_all = x @ Wk + bk ; v_all = x @ Wv + bv
    q = gather(q_all, query_idx)        # (B, M, D) selected query rows
    att = softmax(mask(q k^T / sqrt(hd)))   # per-query causal mask t <= qidx[m]
    y = (att v) @ Wo + bo

Shapes: B=4, T=4096, D=2048, n_head=16, hd=128, M=512.

Sharding (8 cores): core = 2*b + g  handles batch b and head-group g
(8 heads = 1024 feature cols).  Q/K/V projections are column-parallel,
out-proj is row-parallel; the two partial outputs per batch are summed
on the host.  All matmul inputs are bf16 (fp32 PSUM accumulation).

Host-side prep per core: transpose x/xq to (D, T) layout (the PE needs
the contraction dim on partitions), gather the M query rows of x,
slice/cast weights, precompute the additive causal mask, and compute
per-t-chunk skip bounds from query_idx so fully-masked regions of the
score matrix are never computed (~45% of attention work skipped for
sorted indices; correct for arbitrary indices).

Per-core schedule: a HAM-warmup block of dummy matmuls runs during the
initial DMA wait so the PE activity monitor un-throttles (1.2->2.4 GHz)
before the first real matmul; then Q projection (small, covers the K/V
weight prefetch -- DMA issue order is consumption order and the first
x chunk is prefetched during it), then one fused pass over x computing
K^T (head 0 kept in SBUF, heads 1-7 streamed to a DRAM scratch) and V
(resident in SBUF), then per-head attention with K^T streamed back in
(prefetched a full head ahead, before the out-proj weights, so the
in-order sync queue can't starve it), then the output projection
(both output halves of a group share one SBUF tile -> one DMA trigger,
partials written as bf16).
The attention inner loop runs as a flat (head, batch) software
pipeline: the P@V matmuls lag the score/exp stream by TWO batches and
the row-sum matmuls by THREE (their esum inputs come off the more
backlogged DVE FIFO), both ACROSS head boundaries, and the oldest
pending batch drains BEFORE each batch's scores, so the (strict
program order) tensor engine always has ready work when the exp
stream hiccups.  The exp reads the score psum directly (masking is a
post-exp 0/1 multiply on the bf16 e tile at DVE 4x rate, off the
psum-slot critical path).  Score/exp psum slots: two 2-bank [128,2,M]
supertiles plus a THIRD single-bank half-width slot used by pairs
whose live window fits in [M/2, M) -- the bank freed by packing both
in-flight heads' row-sum accumulators into one bank at partitions
0/32 (the ones-matmul's tile_position col-group derives from the out
AP base partition).  E chunk pairs are pre-summed on the vector
engine in bf16 4x mode, then batch pairs are pre-summed again so the
PE row-sum runs over [lo0,lo1)+[lo1,M) once per batch.
Softmax normalization never touches the PE: 1/l is computed on DVE and
broadcast across partitions on the otherwise-idle GPSIMD engine (whose
Q7 library is preloaded at kernel start), split in two stages a batch
apart so the vector FIFO never waits on GPSIMD.
Measured ~630 us on hardware per NeuronCore (8 cores SPMD), HAM warm
for the whole span, attention phase at ~162 ns/matmul (vs ~660 us for
the session-start baseline).
"""

import sys
import types
from contextlib import ExitStack

import numpy as np
import ml_dtypes

import concourse.bass as bass
import concourse.tile as tile
import concourse.mybir as mybir
from concourse import bacc
from concourse.bass_utils import run_bass_kernel_spmd

BF16 = mybir.dt.bfloat16
F32 = mybir.dt.float32
NPBF = ml_dtypes.bfloat16

B, T, D = 4, 4096, 2048
NH, HD, M = 16, 128, 512
NHG = 8            # heads per core (group)
DG = NHG * HD      # 1024 feature cols per core
NT = T // 128      # 32 t-chunks
ND = D // 128      # 16 d-chunks
MASK_VAL = np.float32(-30000.0)


def _install_ntff_hook():
    """Register the axon NTFF profiling hook if the image's antenv lacks it."""
    try:
        from antenv.axon_hooks import get_axon_ntff_profile_hook  # noqa: F401
        return
    except ImportError:
        pass
    try:
        import antenv
        from trn_agent_boot.trn_boot import _ntff_profile_via_ctypes

        mod = types.ModuleType("antenv.axon_hooks")
        hook = [None]
        mod.set_axon_ntff_profile_hook = lambda h: hook.__setitem__(0, h)
        mod.get_axon_ntff_profile_hook = lambda: hook[0]
        sys.modules["antenv.axon_hooks"] = mod
        antenv.axon_hooks = mod
        mod.set_axon_ntff_profile_hook(
            _ntff_profile_via_ctypes("/opt/axon/libaxon_pjrt.so")
        )
    except Exception:
        pass


def build_program(flo, fhi):
    """Build the per-core Bass program.

    flo[i]: first m column with any allowed key in t-chunk i (cols below
            are fully masked there -> never computed).
    fhi[i]: first m column fully allowed in t-chunk i (cols beyond need
            no mask add).
    Both are unions over the 4 batches so one program serves all cores.
    """
    nc = bacc.Bacc("TRN2", target_bir_lowering=False, debug=False)

    xT = nc.dram_tensor("xT", [D, T], BF16, kind="ExternalInput")
    xqT = nc.dram_tensor("xqT", [D, M], BF16, kind="ExternalInput")
    wk = nc.dram_tensor("wk", [D, DG], BF16, kind="ExternalInput")
    wv = nc.dram_tensor("wv", [D, DG], BF16, kind="ExternalInput")
    wq = nc.dram_tensor("wq", [D, DG], BF16, kind="ExternalInput")
    wo = nc.dram_tensor("wo", [DG, D], BF16, kind="ExternalInput")
    maskd = nc.dram_tensor("mask", [T, M], BF16, kind="ExternalInput")
    bks = nc.dram_tensor("bks", [128, NHG], F32, kind="ExternalInput")
    bqs = nc.dram_tensor("bqs", [128, NHG], F32, kind="ExternalInput")
    y = nc.dram_tensor("y", [M, D], BF16, kind="ExternalOutput")

    # (c*128+p, t) views for 4-chunk batched DMA
    xTr = xT.rearrange("(c p) t -> p c t", p=128)
    xqTr = xqT.rearrange("(c p) t -> p c t", p=128)
    wkr = wk.rearrange("(c p) t -> p c t", p=128)
    wvr = wv.rearrange("(c p) t -> p c t", p=128)
    wqr = wq.rearrange("(c p) t -> p c t", p=128)
    wor = wo.rearrange("(c p) t -> p c t", p=128)
    maskr = maskd.rearrange("(c p) t -> p c t", p=128)

    with ExitStack() as ctx:
        tc = ctx.enter_context(tile.TileContext(nc))

        # ---- persistent tiles --------------------------------------
        persist = ctx.enter_context(tc.tile_pool(name="persist", bufs=1))
        v_t = [persist.tile([128, DG], BF16, name=f"v{i}", tag=f"v{i}") for i in range(NT)]
        qt_t = [persist.tile([128, M], BF16, name=f"qt{j}", tag=f"qt{j}") for j in range(NHG)]
        ot_t = [persist.tile([128, M], BF16, name=f"ot{j}", tag=f"ot{j}") for j in range(NHG)]
        bias_k = persist.tile([128, NHG], F32, name="bias_k", tag="bias_k")
        bias_q = persist.tile([128, NHG], F32, name="bias_q", tag="bias_q")
        zbias = persist.tile([128, 1], F32, name="zbias", tag="zbias")
        ones_c = persist.tile([128, 1], BF16, name="ones_c", tag="ones_c")
        ones_r = persist.tile([1, 128], F32, name="ones_r", tag="ones_r")
        kt0_sb = persist.tile([128, T], BF16, name="kt0_sb", tag="kt0_sb")
        # mask super-tiles: 4 t-chunks each, shared col-window
        mlo = [min(flo[4 * g : 4 * g + 4]) for g in range(NT // 4)]
        mhi = [max(fhi[4 * g : 4 * g + 4]) for g in range(NT // 4)]
        mask_t = [
            persist.tile(
                [128, 4, max(mhi[g] - mlo[g], 1)], BF16,
                name=f"mask{g}", tag=f"mask{g}",
            )
            for g in range(NT // 4)
        ]
        dram = ctx.enter_context(tc.tile_pool(name="dram", bufs=1, space="DRAM"))
        ktd = dram.tile([NHG, 128, T], BF16, name="ktd")
        ktd_r = ktd.rearrange("j p t -> p j t")

        nc.sync.dma_start(bias_k[:], bks[:])
        nc.sync.dma_start(bias_q[:], bqs[:])
        nc.vector.memset(zbias[:], 0.0)
        nc.vector.memset(ones_c[:], 1.0)
        nc.vector.memset(ones_r[:], 1.0)

        # ---- HAM warmup: dummy matmuls on a zero tile fill the initial
        # DMA wait so the PE's activity monitor un-throttles (4/8 -> 8/8)
        # before the first real matmul; results are discarded.  A dummy
        # partition_broadcast preloads the GPSIMD Q7 library so the first
        # real one (softmax norm) doesn't pay the lazy library load.
        warmp = ctx.enter_context(tc.tile_pool(name="warm", bufs=1))
        wz = warmp.tile([128, 512], BF16, name="wz", tag="wz")
        gbw = warmp.tile([128, 1], F32, name="gbw", tag="gbw")
        nc.vector.memset(wz[:], 0.0)
        nc.gpsimd.partition_broadcast(gbw[:], zbias[0:1, :], 128)
        with tc.tile_pool(name="pwarm", bufs=1, space="PSUM") as pwp:
            pw = pwp.tile([128, 512], F32, name="pw", tag="pw")
            NWARM = 44
            for r in range(NWARM):
                nc.tensor.matmul(
                    pw[:], wz[:, 0:128], wz[:],
                    start=(r == 0), stop=(r == NWARM - 1),
                    skip_group_check=True,
                )

        with ExitStack() as phaseA:
            # wk prefetch + first x chunk ride under phase A-Q
            wkp = phaseA.enter_context(tc.tile_pool(name="wkp", bufs=1))
            wk_t = [wkp.tile([128, 4, DG], BF16, name=f"wk{d}", tag=f"wk{d}") for d in range(4)]
            xtp = phaseA.enter_context(tc.tile_pool(name="xtp", bufs=2))
            KTS = 512
            xt0 = [xtp.tile([128, 4, KTS], BF16, name=f"xt{d}", tag=f"xt{d}") for d in range(4)]

            # ---- phase A-Q: Qt[j] = ((xq @ wq_j + bq_j)/sqrt(hd))^T ----
            # DMA issue order = consumption order: xq + wq head-pair 0/1
            # first (first matmul group waits ~4 MB), then wk + x chunk 0
            # (needed at phase A-KV entry), then the rest of wq.
            with (
                nc.named_scope("phase_AQ"),
                tc.tile_pool(name="wqp", bufs=1) as wqp,
                tc.tile_pool(name="xqp", bufs=1) as xqp,
                tc.tile_pool(name="pq", bufs=4, space="PSUM") as pqp,
            ):
                xq_t = [xqp.tile([128, 4, M], BF16, name=f"xq{d}", tag=f"xq{d}") for d in range(4)]
                for d in range(4):
                    nc.sync.dma_start(xq_t[d][:], xqTr[:, 4 * d : 4 * d + 4, :])
                # wq in head-pair column slices: (jg, s) -> 4 d-chunks x 256 cols
                wq_t = {}
                for jg in range(4):
                    for s in range(4):
                        wq_t[jg, s] = wqp.tile(
                            [128, 4, 256], BF16, name=f"wq{jg}_{s}", tag=f"wq{jg}_{s}"
                        )
                def _dma_wq(jg):
                    for s in range(4):
                        nc.sync.dma_start(
                            wq_t[jg, s][:],
                            wqr[:, 4 * s : 4 * s + 4, jg * 256 : (jg + 1) * 256],
                        )
                for jg in range(4):
                    _dma_wq(jg)
                for d in range(4):
                    nc.sync.dma_start(wk_t[d][:], wkr[:, 4 * d : 4 * d + 4, :])
                for d in range(4):
                    nc.sync.dma_start(xt0[d][:], xTr[:, 4 * d : 4 * d + 4, 0:KTS])
                inv_s = 1.0 / float(np.sqrt(HD))
                for j in range(NHG):
                    jg, co = j // 2, (j % 2) * 128
                    pq = pqp.tile([128, M], F32, name="pq", tag="pq")
                    for d in range(ND):
                        nc.tensor.matmul(
                            pq[:],
                            wq_t[jg, d // 4][:, d % 4, co : co + 128],
                            xq_t[d // 4][:, d % 4, :],
                            start=(d == 0),
                            stop=(d == ND - 1),
                        )
                    nc.scalar.activation(
                        qt_t[j][:],
                        pq[:],
                        mybir.ActivationFunctionType.Identity,
                        scale=inv_s,
                        bias=bias_q[:, j : j + 1],
                    )

            # ---- phase A-KV: one pass over x computing Kt and V --------
            with (
                nc.named_scope("phase_AKV"),
                tc.tile_pool(name="wvp", bufs=1) as wvp,
                tc.tile_pool(name="kst", bufs=3) as kstp,
                tc.tile_pool(name="pk", bufs=3, space="PSUM") as pkp,
                tc.tile_pool(name="pv", bufs=3, space="PSUM") as pvp,
            ):
                wv_t = [wvp.tile([128, 4, DG], BF16, name=f"wv{d}", tag=f"wv{d}") for d in range(4)]
                for d in range(4):
                    nc.sync.dma_start(wv_t[d][:], wvr[:, 4 * d : 4 * d + 4, :])
            for g in range(NT // 4):
                if mlo[g] < M and mhi[g] > mlo[g]:
                    nc.sync.dma_start(
                        mask_t[g][:, :, : mhi[g] - mlo[g]],
                        maskr[:, 4 * g : 4 * g + 4, mlo[g] : mhi[g]],
                    )
            for ts in range(T // KTS):
                if ts == 0:
                    xt_t = xt0
                else:
                    xt_t = [xtp.tile([128, 4, KTS], BF16, name=f"xt{d}", tag=f"xt{d}") for d in range(4)]
                    for d in range(4):
                        nc.sync.dma_start(
                            xt_t[d][:], xTr[:, 4 * d : 4 * d + 4, ts * KTS : (ts + 1) * KTS]
                        )
                # K^T: per head j, (hd, KTS) tile; staged 4 heads per DMA
                for jg in range(2):
                    ks = kstp.tile([128, 4, KTS], BF16, name="ks", tag="ks")
                    for jj in range(4):
                        j = 4 * jg + jj
                        pk = pkp.tile([128, KTS], F32, name="pk", tag="pk")
                        for d in range(ND):
                            nc.tensor.matmul(
                                pk[:],
                                wk_t[d // 4][:, d % 4, j * 128 : (j + 1) * 128],
                                xt_t[d // 4][:, d % 4, :],
                                start=(d == 0),
                                stop=(d == ND - 1),
                            )
                        nc.scalar.activation(
                            kt0_sb[:, ts * KTS : (ts + 1) * KTS] if j == 0
                            else ks[:, jj, :],
                            pk[:],
                            mybir.ActivationFunctionType.Identity,
                            bias=bias_k[:, j : j + 1],
                        )
                    nc.sync.dma_start(
                        ktd_r[:, 4 * jg : 4 * jg + 4, ts * KTS : (ts + 1) * KTS],
                        ks[:],
                    )
                # V: (t, DG) tiles
                for u in range(KTS // 128):
                    i = ts * (KTS // 128) + u
                    for f in range(2):
                        pv = pvp.tile([128, 512], F32, name="pv", tag="pv")
                        for d in range(ND):
                            nc.tensor.matmul(
                                pv[:],
                                xt_t[d // 4][:, d % 4, u * 128 : (u + 1) * 128],
                                wv_t[d // 4][:, d % 4, f * 512 : (f + 1) * 512],
                                start=(d == 0),
                                stop=(d == ND - 1),
                            )
                        nc.vector.tensor_copy(
                            v_t[i][:, f * 512 : (f + 1) * 512], pv[:]
                        )

        # ---- phase B prefetch: out-proj weights (issued after the
        # first K^T head prefetch so kth[1] isn't queued behind 4 MB)
        wop = ctx.enter_context(tc.tile_pool(name="wop", bufs=1))
        wo_t = [wop.tile([128, 4, D], BF16, name=f"wo{d}", tag=f"wo{d}") for d in range(2)]

        # ---- phase B: attention per head, 4-chunk batched ----------
        chunks = [i for i in range(NT) if flo[i] < M]
        pairs = [chunks[k : k + 2] for k in range(0, len(chunks), 2)]
        batches = [pairs[k : k + 2] for k in range(0, len(pairs), 2)]
        with (
            nc.named_scope("phase_B"),
            tc.tile_pool(name="kth", bufs=3) as kthp,
            tc.tile_pool(name="ps", bufs=2, space="PSUM") as psp,
            tc.tile_pool(name="pssm", bufs=1, space="PSUM") as pssm,
            tc.tile_pool(name="po", bufs=2, space="PSUM") as pop,
            tc.tile_pool(name="pl", bufs=1, space="PSUM") as plp,
            tc.tile_pool(name="esb", bufs=8) as esb,
            tc.tile_pool(name="lsb", bufs=2) as lsb,
        ):
            po_q, pl_q, lb_q = {}, {}, {}

            def norm_stage1(j):
                """1/l for head j: DVE reciprocal + GPSIMD partition
                broadcast.  No PE instruction is involved, and the final
                multiply is deferred a batch (norm_stage2) so the Vector
                FIFO never sits waiting on the GPSIMD broadcast."""
                pl = pl_q.pop(j)
                l_sb = lsb.tile([1, M], F32, name="l", tag="l")
                linv = lsb.tile([1, M], F32, name="linv", tag="linv")
                nc.vector.tensor_copy(l_sb[:], pl[0:1, :])
                nc.vector.reciprocal_approx_fast(linv[:], l_sb[:])
                lb_sb = lsb.tile([128, M], F32, name="lb", tag="lb")
                nc.gpsimd.partition_broadcast(lb_sb[:], linv[:], 128)
                lb_q[j] = lb_sb

            def norm_stage2(j):
                po, lb_sb = po_q.pop(j), lb_q.pop(j)
                nc.vector.tensor_mul(ot_t[j][:], po[:], lb_sb[:])

            kth = {0: kt0_sb}
            # prefetch head 1's K^T first, then the (big) out-proj weights
            kth[1] = kthp.tile([128, T], BF16, name="kth", tag="kth")
            nc.sync.dma_start(kth[1][:], ktd[1])
            for d in range(2):
                nc.sync.dma_start(wo_t[d][:], wor[:, 4 * d : 4 * d + 4, :])
            state = {}  # j -> [po_start_pending, l_start_pending]

            def drain_pv(pj, cur, last_b):
                """Emit the lagged PV matmuls for head pj's batch."""
                st = state[pj]
                for k, (pair, e, lo) in enumerate(cur):
                    for u, i in enumerate(pair):
                        nc.tensor.matmul(
                            po_q[pj][:, lo:M],
                            v_t[i][:, pj * 128 : (pj + 1) * 128],
                            e[:, u, lo:M],
                            start=st[0],
                            stop=(last_b and k == len(cur) - 1 and u == len(pair) - 1),
                            skip_group_check=True,
                        )
                        st[0] = False

            def drain_rs(pj, rsums, last_b):
                """Emit the (further-lagged) row-sum matmuls: their esum2
                inputs come off the more-backlogged DVE FIFO, so they get
                an extra batch of slack vs the PV drains."""
                st = state[pj]
                for k, (rs, lo, hi) in enumerate(rsums):
                    nc.tensor.matmul(
                        pl_q[pj][0:1, lo:hi], ones_c[:], rs,
                        start=st[1], stop=(last_b and k == len(rsums) - 1),
                        skip_group_check=True,
                    )
                    st[1] = False

            # flat (head, batch) pipeline: PV/l matmuls lag the S/exp
            # stream by TWO batches ACROSS head boundaries, so the PE has
            # queued drain work to chew whenever the exp stream hiccups
            pend_q = []  # [(j, cur, is_last_batch_of_head), ...]
            rs_q = []    # row-sum batches, drained one batch later
            # both in-flight heads' l accumulators share ONE psum bank:
            # head j at partition 32*(j%2) (the row-sum matmul's
            # tile_position col-group derives from the out base partition)
            pl2 = plp.tile([64, M], F32, name="pl2", tag="pl2")
            pair_ctr, last_small = [0], [-2]
            for j in range(NHG):
                po_q[j] = pop.tile([128, M], F32, name="po", tag="po")
                pl_q[j] = pl2[32 * (j % 2) : 32 * (j % 2) + 1, :]
                state[j] = [True, True]
                for bi, batch in enumerate(batches):
                    # drain the oldest pending batches FIRST: their inputs
                    # are long ready, so the PE (strict program order)
                    # chews them while this batch's scores wait on
                    # psum-slot recycling at head boundaries
                    if len(pend_q) >= 2:
                        drain_pv(*pend_q.pop(0))
                    if len(rs_q) >= 3:
                        drain_rs(*rs_q.pop(0))
                    cur = []
                    for pair in batch:
                        lo_min = min(flo[i] for i in pair)
                        # pairs whose live window fits in half a bank can
                        # use the single-bank half-slot, giving the score/
                        # exp pipeline a 3rd slot for the tail of each head
                        use_small = lo_min >= M // 2 and pair_ctr[0] - last_small[0] >= 2
                        if use_small:
                            pst = pssm.tile([128, 2, M // 2], F32, name="psts", tag="pssm")
                            off = M // 2
                            last_small[0] = pair_ctr[0]
                        else:
                            pst = psp.tile([128, 2, M], F32, name="pst", tag="ps")
                            off = 0
                        pair_ctr[0] += 1
                        for u, i in enumerate(pair):
                            nc.tensor.matmul(
                                pst[:, u, lo_min - off : M - off],
                                kth[j][:, i * 128 : (i + 1) * 128],
                                qt_t[j][:, lo_min:M],
                                start=True,
                                stop=True,
                                skip_group_check=True,
                            )
                        # exp runs straight off the score matmul (no DVE op
                        # in between, so the psum slot frees at Scalar speed);
                        # masking is a 0/1 multiply on the bf16 e tile after,
                        # off the slot-recycle critical path.  Cols beyond a
                        # chunk's own [lo, fhi) window multiply by 1 or touch
                        # lanes the narrower chunk never reads.
                        fhi_max = max(fhi[i] for i in pair)
                        g = pair[0] // 4
                        um = pair[0] % 4
                        e = esb.tile([128, 2, M], BF16, name="e", tag="e")
                        nc.scalar.activation(
                            e[:, : len(pair), lo_min:M],
                            pst[:, : len(pair), lo_min - off : M - off],
                            mybir.ActivationFunctionType.Exp,
                            bias=zbias[:],
                        )
                        if lo_min < fhi_max:
                            nc.vector.tensor_mul(
                                e[:, : len(pair), lo_min:fhi_max],
                                e[:, : len(pair), lo_min:fhi_max],
                                mask_t[g][:, um : um + len(pair), lo_min - mlo[g] : fhi_max - mlo[g]],
                            )
                        if len(pair) == 2:
                            # pair-sum on DVE (bf16 4x) so the PE does one
                            # row-sum matmul per pair instead of per chunk
                            esum = esb.tile([128, M], BF16, name="esum", tag="esum", bufs=6)
                            nc.vector.tensor_add(
                                esum[:, lo_min:M],
                                e[:, 0, lo_min:M],
                                e[:, 1, lo_min:M],
                            )
                            cur.append((pair, e, esum, lo_min))
                        else:
                            cur.append((pair, e, None, lo_min))
                    # batch-level pre-sum: the two pair esums are summed on
                    # DVE over the second pair's window so the PE row-sum
                    # runs once over [lo0, lo1) and once over [lo1, M)
                    # instead of twice over nearly-full windows.
                    rsums = []
                    if (
                        len(cur) == 2
                        and cur[0][2] is not None
                        and cur[1][2] is not None
                    ):
                        (pa, ea, esa, loa), (pb, eb, esb_, lob) = sorted(
                            cur, key=lambda c: c[3]
                        )
                        esum2 = esb.tile([128, M], BF16, name="esum2", tag="esum2", bufs=5)
                        nc.vector.tensor_add(
                            esum2[:, lob:M], esa[:, lob:M], esb_[:, lob:M]
                        )
                        if loa < lob:
                            rsums.append((esa[:, loa:lob], loa, lob))
                        rsums.append((esum2[:, lob:M], lob, M))
                    else:
                        for (pair, e, esum, lo) in cur:
                            rs = esum[:, lo:M] if esum is not None else e[:, 0, lo:M]
                            rsums.append((rs, lo, M))
                    pv_cur = [(pair, e, lo) for (pair, e, esum, lo) in cur]
                    pend_q.append((j, pv_cur, bi == len(batches) - 1))
                    rs_q.append((j, rsums, bi == len(batches) - 1))
                    if bi == 0 and j >= 1 and j + 1 < NHG:
                        # prefetch next head's K^T (a full head of lead time)
                        kth[j + 1] = kthp.tile([128, T], BF16, name="kth", tag="kth")
                        nc.sync.dma_start(kth[j + 1][:], ktd[j + 1])
                    if bi == 2 and j > 0 and (j - 1) in pl_q:
                        norm_stage1(j - 1)
                    if bi == 3 and j > 0 and (j - 1) in lb_q:
                        norm_stage2(j - 1)
            while pend_q or rs_q:
                if pend_q:
                    drain_pv(*pend_q.pop(0))
                if rs_q:
                    drain_rs(*rs_q.pop(0))
            norm_stage1(NHG - 1)
            norm_stage2(NHG - 1)

        # ---- phase C: y = O @ wo  (row-parallel partial) -----------
        with (
            nc.named_scope("phase_C"),
            tc.tile_pool(name="py", bufs=2, space="PSUM") as pyp,
            tc.tile_pool(name="ysb", bufs=3) as ysb,
        ):
            # fo pairs share the stationary ot slice -> one weight load
            # feeds two 512-wide matmuls
            for mb in range(M // 128):
                for fp in range(D // 1024):
                    py = [
                        pyp.tile([128, 512], F32, name="py", tag=f"py{h}")
                        for h in range(2)
                    ]
                    for j in range(NHG):
                        for h in range(2):
                            fo = 2 * fp + h
                            nc.tensor.matmul(
                                py[h][:],
                                ot_t[j][:, mb * 128 : (mb + 1) * 128],
                                wo_t[j // 4][:, j % 4, fo * 512 : (fo + 1) * 512],
                                start=(j == 0),
                                stop=(j == NHG - 1),
                                skip_group_check=True,
                            )
                    # both halves copied into one tile -> one DMA trigger
                    ys = ysb.tile([128, 1024], BF16, name="ys", tag="ys")
                    for h in range(2):
                        nc.scalar.copy(ys[:, h * 512 : (h + 1) * 512], py[h][:])
                    nc.sync.dma_start(
                        y[
                            mb * 128 : (mb + 1) * 128,
                            fp * 1024 : (fp + 1) * 1024,
                        ],
                        ys[:],
                    )

    nc.compile()
    return nc


_cache = {}


def _get_program(flo, fhi):
    key = (tuple(flo), tuple(fhi))
    if key not in _cache:
        _cache[key] = build_program(list(flo), list(fhi))
    return _cache[key]


def _prep(inputs):
    x = np.asarray(inputs["x"], dtype=np.float32)
    qidx = np.asarray(inputs["query_idx"]).astype(np.int64)
    Wq = np.asarray(inputs["Wq"], dtype=np.float32)
    Wk = np.asarray(inputs["Wk"], dtype=np.float32)
    Wv = np.asarray(inputs["Wv"], dtype=np.float32)
    Wo = np.asarray(inputs["Wo"], dtype=np.float32)
    bq = np.asarray(inputs["bq"], dtype=np.float32)
    bk = np.asarray(inputs["bk"], dtype=np.float32)
    bv = np.asarray(inputs["bv"], dtype=np.float32)
    bo = np.asarray(inputs["bo"], dtype=np.float32)

    # Per-t-chunk skip bounds, union over batches.  flo[i] = first m that
    # attends into chunk i (everything below is fully masked there);
    # fhi[i] = one past the last m only partially covered by chunk i.
    # Computed positionally so they are correct even for unsorted
    # query_idx (just less effective at skipping).
    flo = [M] * NT
    fhi = [0] * NT
    for b in range(B):
        for i in range(NT):
            allowed = qidx[b] >= 128 * i          # chunk i not fully masked
            partial = qidx[b] < 128 * (i + 1)     # chunk i not fully allowed
            lo_b = int(np.argmax(allowed)) if allowed.any() else M
            hi_b = M - int(np.argmax(partial[::-1])) if partial.any() else 0
            flo[i] = min(flo[i], lo_b)
            fhi[i] = max(fhi[i], hi_b)

    in_maps = []
    tgrid = np.arange(T)[:, None]
    for core in range(8):
        b, g = divmod(core, 2)
        sl = slice(g * DG, (g + 1) * DG)
        xb = x[b]
        mask = np.where(tgrid <= qidx[b][None, :], np.float32(1), np.float32(0))
        in_maps.append(
            {
                "xT": np.ascontiguousarray(xb.T.astype(NPBF)),
                "xqT": np.ascontiguousarray(xb[qidx[b]].T.astype(NPBF)),
                "wk": np.ascontiguousarray(Wk[:, sl].astype(NPBF)),
                "wv": np.ascontiguousarray(Wv[:, sl].astype(NPBF)),
                "wq": np.ascontiguousarray(Wq[:, sl].astype(NPBF)),
                "wo": np.ascontiguousarray(Wo[sl, :].astype(NPBF)),
                "mask": np.ascontiguousarray(mask.astype(NPBF)),
                "bks": np.ascontiguousarray(bk[sl].reshape(NHG, 128).T),
                "bqs": np.ascontiguousarray(
                    (bq[sl] / np.sqrt(HD)).reshape(NHG, 128).T.astype(np.float32)
                ),
            }
        )

    const = (bv.astype(np.float64) @ Wo.astype(np.float64) + bo).astype(np.float32)
    return flo, fhi, in_maps, const


def run(inputs, trace=False, trace_kwargs=None):
    _install_ntff_hook()
    flo, fhi, in_maps, const = _prep(inputs)
    nc = _get_program(flo, fhi)
    res = run_bass_kernel_spmd(
        nc, in_maps, list(range(8)), trace=trace, **(trace_kwargs or {})
    )
    out = np.zeros((B, M, D), dtype=np.float32)
    for b in range(B):
        out[b] = (
            np.asarray(res.results[2 * b]["y"], dtype=np.float32)
            + np.asarray(res.results[2 * b + 1]["y"], dtype=np.float32)
            + const
        )
    return out, res


def kernel(**inputs) -> np.ndarray:
    out, _ = run(inputs, trace=False)
    return out

